# revision 31
# baseline (speedup 1.0000x reference)
"""Trainium2 Bass kernel for nn_BBPMAssociativeModel.

Model: per-batch associative memory — pairs (key, value-token) from the
input sequence are scatter-added into a 8192-slot memory via 4 hash
probes, the memory is read back at the query token's 4 probe slots,
and the mean read vector goes through a [D, V] classifier.

Algebraic collapse used here: the memory is never materialized.
    r_b = sum_p (m_{b,p} / K) * emb_table[x[b, 2p+1]]
where m_{b,p} = |{(k,k') : probe(key_{b,p})[k'] == probe(query_b)[k]}|.
Since probes land in 8192 slots, m is almost always 0 — only a handful
of (b, p) pairs contribute. The host computes the integer hash/match
part (index math only), and the device does all floating-point work:
    rT = rows.T @ CT          (gathered embedding rows x coefficients)
    logits = rT.T @ W.T + b   (vocab-sharded over 8 cores)

Per-core device program (vocab shard of 4000 columns):
  - rows  [E, 544]  fp16 gathered embedding rows | coefficient rows,
                    so phase 1's whole input arrives in one DMA
  - wt    [512, 4000] W.T shard (fp16 stream by default — halves the
                    memory-bound W traffic; logits stay fp32-accumulated)
  - bias  [1, 4000] b shard (variant only emitted when b is nonzero)
  - out   [32, 4000] logits shard (fp32)
"""

import numpy as np
from contextlib import ExitStack

B, T, D, V = 32, 2048, 512, 32000
NCORES = 8
VS = V // NCORES        # 4000 vocab columns per core
NUM_SLOTS, KP = 8192, 4
SEED = np.uint32(1234)
GOLD = np.uint32(0x9E3779B9)
KC = D // 128           # 4 contraction chunks
NTW = 500               # matmul moving free dim (one PSUM bank of fp32)
NT = VS // NTW          # 8 n-tiles per core
E_DEFAULT = 128

# W-stream dtype: "f16" halves DMA traffic (fp16 mantissa keeps the
# logit error ~5e-4 relative); "f32r" is the full-precision-stream mode.
W_DTYPE = "f16"

_prog_cache = {}
LAST_RESULTS = None     # stashed BassKernelResults (for profiling in test.py)


def _mix32(h):
    h = h.astype(np.uint32, copy=False)
    h = h ^ (h >> np.uint32(16))
    h = h * np.uint32(0x85EBCA6B)
    h = h ^ (h >> np.uint32(13))
    h = h * np.uint32(0xC2B2AE35)
    h = h ^ (h >> np.uint32(16))
    return h


def _probe_slots(tok):
    hx = _mix32(tok.astype(np.uint32) ^ SEED)
    offs = np.arange(KP, dtype=np.uint32) * GOLD
    return (_mix32(hx[..., None] + offs) % np.uint32(NUM_SLOTS)).astype(np.int32)


def _split_multi_waits(nc, limit=1):
    """The nix-baked walrus rejects instructions with more than `limit`
    sem-waits ("Too many sync wait commands", CoreV3GenImpl setupSyncWait).
    Hoist extra waits onto single-wait NOPs preceding the instruction on
    the same engine (waiting earlier on the same engine is always safe)."""
    import concourse.mybir as mybir

    for fn in nc.m.functions:
        for bb in fn.blocks:
            new_insts = []
            for ins in bb.instructions:
                si = ins.sync_info
                if si is not None and len(si.on_wait) > limit:
                    waits = list(si.on_wait)
                    extra, keep = waits[:-limit], waits[-limit:]
                    for idx, w in enumerate(extra):
                        new_insts.append(mybir.InstNoOp(
                            name=f"{ins.name}-wsplit{idx}",
                            sync_info=mybir.SyncInfo(on_wait=[w], on_update=[]),
                            bass_nofuse=True,
                            engine=ins.engine,
                        ))
                    ins.sync_info = mybir.SyncInfo(
                        on_wait=keep, on_update=list(si.on_update))
                new_insts.append(ins)
            bb.instructions[:] = new_insts


def _strip_entry_barrier(nc):
    """Remove the entry-BB all-engine boot barrier and the const-tile
    memsets (walrus flags those consts as having no readers). The barrier
    only serializes engine boot: every real dependency in the body is
    carried by Tile-generated semaphores, and the event-semaphore
    barrier instances are self-resetting, so the exit barriers are
    unaffected. This lets each engine (notably the DMA-trigger engines)
    start its body work as soon as it boots instead of waiting ~3us for
    the slowest engine."""
    import concourse.mybir as mybir

    def _is_barrier(ins):
        if not isinstance(ins, (mybir.InstDrain, mybir.InstEventSemaphore)):
            return False
        si = ins.sync_info
        names = [w.ant_name for w in (si.on_wait if si else [])]
        names += [getattr(u, "ant_name", "") or ""
                  for u in (si.on_update if si else [])]
        return any(n.startswith("barrier_") for n in names) or not names

    bb = nc.m.functions[0].blocks[0]
    bb.instructions[:] = [
        ins for ins in bb.instructions
        if not (isinstance(ins, mybir.InstMemset) or _is_barrier(ins))
    ]




def _build(E, has_bias, wdt=None, split=True):
    import concourse.bass as bass
    import concourse.mybir as mybir
    from concourse.bass import MemorySpace
    from concourse.tile import TileContext

    if wdt is None:
        wdt = W_DTYPE
    f32 = mybir.dt.float32
    # float32r: same 4-byte fp32 layout, but the PE runs a single-pass
    # matmul (vs 2-pass FP32HI/FP32LO for plain fp32) at ~2x throughput
    # with slightly reduced internal precision.
    f32r = mybir.dt.float32r
    fw = mybir.dt.float16 if wdt == "f16" else f32r
    EC = E // 128
    nc = bass.Bass(monotonic_sem_count=0, enable_partition_id=False)
    # rows buffer: [E, D + B] — embedding row (D cols) | ct row (B cols),
    # merged so the whole phase-1 input arrives in ONE well-shaped DMA.
    rows = nc.declare_dram_parameter("rows", [E, D + B], fw, isOutput=False)
    wt = nc.declare_dram_parameter("wt", [D, VS], fw, isOutput=False)
    if has_bias:
        bias = nc.declare_dram_parameter("bias", [1, VS], f32, isOutput=False)
    out = nc.declare_dram_parameter("out", [B, VS], f32, isOutput=True)

    with TileContext(nc) as tc:
        with ExitStack() as ctx:
            const = ctx.enter_context(tc.tile_pool(name="const", bufs=1))
            rows_sb = const.tile([128, EC, D + B], fw)
            nc.gpsimd.dma_start(
                rows_sb[:], rows.rearrange("(n p) d -> p n d", p=128))
            if has_bias:
                bias_sb = const.tile([1, VS], f32)
                ones_sb = const.tile([1, B], f32)
                nc.sync.dma_start(bias_sb[:], bias[:])
                nc.any.memset(ones_sb[:], 1.0)

            wtp = ctx.enter_context(tc.tile_pool(name="wtp", bufs=16))
            obuf = ctx.enter_context(tc.tile_pool(name="obuf", bufs=NT))
            with tc.tile_pool(name="mpsum", bufs=NT, space=MemorySpace.PSUM) as mpsum:
                # PE warm-up: the HAM clock gate keeps the PE at 1.2 GHz
                # until it has seen ~3.4us of sustained matmul activity.
                # Run dummy matmuls on a zeroed tile while the first W
                # chunks are still in flight so the real matmuls start
                # at 2.4 GHz.
                NWARM = 14
                dumw = const.tile([128, 640], fw, name="dumw")
                nc.any.memset(dumw[:], 0.0)
                dps = mpsum.tile([128, 512], f32, name="ps")
                for i in range(NWARM):
                    nc.tensor.matmul(
                        dps[:],
                        dumw[:, :128],
                        dumw[:, 128:640],
                        start=True,
                        stop=True,
                    )

                # Phase 1: rT_k [128, 32] = rows[:, kchunk].T @ CT, k = 0..3
                # (reuses the same PSUM slots the big matmul uses later)
                rt_sb = []
                for k in range(KC):
                    rt_ps = mpsum.tile([128, B], f32, name="ps")
                    for e in range(EC):
                        nc.tensor.matmul(
                            rt_ps[:],
                            rows_sb[:, e, k * 128:(k + 1) * 128],
                            rows_sb[:, e, D:D + B],
                            start=(e == 0),
                            stop=(e == EC - 1),
                        )
                    rt_k = const.tile([128, B], fw, name=f"rt{k}")
                    nc.vector.tensor_copy(rt_k[:], rt_ps[:])
                    rt_sb.append(rt_k)

                # Phase 2: out[:, j*500:(j+1)*500] = rT.T @ wt_j (+ bias_j)
                # W stream: 512 KB transfers (4 KB contiguous per
                # partition) alternating between the sync- and scalar-
                # engine HWDGE queues so one queue's descriptor
                # generation hides under the other's data phase. All
                # tiles are SBUF-resident (bufs = #tiles) so the stream
                # never stalls on slot recycling. Output copies/stores
                # are interleaved into the last k-chunk.
                NW = (2 if wdt != "f16" else 4) * NTW
                NQT = VS // NW
                psums = [
                    mpsum.tile([B, NTW], f32, name="ps") for _ in range(NT)
                ]
                dma_engs = [nc.sync, nc.scalar]
                n_dma = 0
                for k in range(KC):
                    # Finer transfers on the last k-chunk: its completion
                    # semaphores gate the kernel tail, so smaller pieces
                    # start the final matmul/copy/store chain earlier.
                    if k == KC - 1:
                        col_chunks = [NW, NW // 2, NW // 4, NW // 4]
                    else:
                        col_chunks = [NW] * NQT
                    col0 = 0
                    for cw in col_chunks:
                        wq = wtp.tile([128, cw], fw, name="wq",
                                      padded_shape=[128, NW])
                        eng = dma_engs[n_dma % 2]
                        n_dma += 1
                        eng.dma_start(
                            wq[:],
                            wt[k * 128:(k + 1) * 128, col0:col0 + cw],
                        )
                        for jj in range(cw // NTW):
                            j = (col0 + jj * NTW) // NTW
                            nc.tensor.matmul(
                                psums[j][:],
                                rt_sb[k][:],
                                wq[:, jj * NTW:(jj + 1) * NTW],
                                start=(k == 0),
                                stop=(k == KC - 1 and not has_bias),
                            )
                            if k == KC - 1:
                                if has_bias:
                                    nc.tensor.matmul(
                                        psums[j][:],
                                        ones_sb[:],
                                        bias_sb[:, j * NTW:(j + 1) * NTW],
                                        start=False,
                                        stop=True,
                                    )
                                ob = obuf.tile([B, NTW], f32, name="ob")
                                if j == NT - 1:
                                    # Final tile: halve the copy across
                                    # DVE+ACT and store the halves on two
                                    # queues so the last store (and its
                                    # DRAM write receipt, which gates the
                                    # kernel tail) starts sooner.
                                    h = NTW // 2
                                    nc.vector.tensor_copy(
                                        ob[:, :h], psums[j][:, :h])
                                    nc.scalar.copy(
                                        ob[:, h:], psums[j][:, h:])
                                    nc.sync.dma_start(
                                        out[:, j * NTW:j * NTW + h],
                                        ob[:, :h])
                                    # ACT stores its own half: no cross-
                                    # engine handoff, and gpsimd's trigger
                                    # queue (4 earlier stores deep) is
                                    # bypassed for the last write.
                                    nc.scalar.dma_start(
                                        out[:, j * NTW + h:(j + 1) * NTW],
                                        ob[:, h:])
                                elif j % 2 == 0:
                                    nc.vector.tensor_copy(ob[:], psums[j][:])
                                    nc.gpsimd.dma_start(
                                        out[:, j * NTW:(j + 1) * NTW], ob[:])
                                else:
                                    nc.scalar.copy(ob[:], psums[j][:])
                                    nc.sync.dma_start(
                                        out[:, j * NTW:(j + 1) * NTW], ob[:])
                        col0 += cw
    if split:
        _split_multi_waits(nc)
        _strip_entry_barrier(nc)
    return nc


def _get_prog(E, has_bias):
    key = (E, has_bias, W_DTYPE)
    if key not in _prog_cache:
        _prog_cache[key] = _build(E, has_bias)
    return _prog_cache[key]


def _host_prep(x, emb_table):
    """Integer hash/match preprocessing -> packed rows [E, D + B]."""
    ts = np.arange(0, T - 1, 2)
    ts = ts[ts + 1 < T - 1]                      # [P]
    wslots = _probe_slots(x[:, ts])              # [B, P, K]
    qslots = _probe_slots(x[:, -1])              # [B, K]
    m = (wslots[:, :, None, :] == qslots[:, None, :, None]).sum(
        axis=(2, 3), dtype=np.int32)             # [B, P]
    bs, ps = np.nonzero(m)
    n_ent = len(bs)
    E = max(E_DEFAULT, ((n_ent + 127) // 128) * 128)
    rows = np.zeros((E, D + B), np.float32)      # emb row | ct row
    tok = x[:, ts + 1][bs, ps]                   # value tokens of hits
    rows[:n_ent, :D] = emb_table[tok]
    rows[np.arange(n_ent), D + bs] = m[bs, ps].astype(np.float32) / KP
    return rows


def kernel(x, emb_table, W, b):
    global LAST_RESULTS
    from concourse.bass_utils import run_bass_kernel_spmd

    x = np.asarray(x)
    emb_table = np.ascontiguousarray(np.asarray(emb_table, np.float32))
    W = np.asarray(W, np.float32)
    b = np.asarray(b, np.float32)

    rows = _host_prep(x, emb_table)
    has_bias = bool(np.any(b))
    wdt_np = np.float16 if W_DTYPE == "f16" else np.float32
    wt_full = np.ascontiguousarray(W.T.astype(wdt_np))   # [D, V]

    nc = _get_prog(rows.shape[0], has_bias)
    in_maps = []
    for c in range(NCORES):
        m = {
            "rows": rows.astype(wdt_np),
            "wt": np.ascontiguousarray(wt_full[:, c * VS:(c + 1) * VS]),
        }
        if has_bias:
            m["bias"] = np.ascontiguousarray(b[c * VS:(c + 1) * VS]).reshape(1, VS)
        in_maps.append(m)

    res = None
    for attempt in range(3):
        try:
            res = run_bass_kernel_spmd(
                nc, in_maps, core_ids=list(range(NCORES)))
            break
        except Exception:
            # The axon-tunneled device occasionally reports a transient
            # NRT_EXEC_UNIT_UNRECOVERABLE on back-to-back NEFF loads;
            # a re-dispatch on the next attempt succeeds.
            if attempt == 2:
                raise
            import time
            time.sleep(2.0)
    LAST_RESULTS = res

    logits = np.empty((B, V), np.float32)
    for c in range(NCORES):
        logits[:, c * VS:(c + 1) * VS] = res.results[c]["out"]
    return logits


# revision 32
# speedup vs baseline: 1.0623x; 1.0623x over previous
"""Trainium2 Bass kernel for nn_BBPMAssociativeModel.

Model: per-batch associative memory — pairs (key, value-token) from the
input sequence are scatter-added into a 8192-slot memory via 4 hash
probes, the memory is read back at the query token's 4 probe slots,
and the mean read vector goes through a [D, V] classifier.

Algebraic collapse used here: the memory is never materialized.
    r_b = sum_p (m_{b,p} / K) * emb_table[x[b, 2p+1]]
where m_{b,p} = |{(k,k') : probe(key_{b,p})[k'] == probe(query_b)[k]}|.
Since probes land in 8192 slots, m is almost always 0 — only a handful
of (b, p) pairs contribute. The host computes the integer hash/match
part (index math only), and the device does all floating-point work:
    rT = rows.T @ CT          (gathered embedding rows x coefficients)
    logits = rT.T @ W.T + b   (vocab-sharded over 8 cores)

Per-core device program (vocab shard of 4000 columns):
  - rows  [E, 544]  fp16 gathered embedding rows | coefficient rows,
                    so phase 1's whole input arrives in one DMA
  - wt    [512, 4000] W.T shard (fp16 stream by default — halves the
                    memory-bound W traffic; logits stay fp32-accumulated)
  - bias  [1, 4000] b shard (variant only emitted when b is nonzero)
  - out   [32, 4000] logits shard (fp32)
"""

import numpy as np
from contextlib import ExitStack

B, T, D, V = 32, 2048, 512, 32000
NCORES = 8
VS = V // NCORES        # 4000 vocab columns per core
NUM_SLOTS, KP = 8192, 4
SEED = np.uint32(1234)
GOLD = np.uint32(0x9E3779B9)
KC = D // 128           # 4 contraction chunks
NTW = 500               # matmul moving free dim (one PSUM bank of fp32)
NT = VS // NTW          # 8 n-tiles per core
E_DEFAULT = 128

# W-stream dtype: "f16" halves DMA traffic (fp16 mantissa keeps the
# logit error ~5e-4 relative); "f32r" is the full-precision-stream mode.
W_DTYPE = "f16"

_prog_cache = {}
LAST_RESULTS = None     # stashed BassKernelResults (for profiling in test.py)


def _mix32(h):
    h = h.astype(np.uint32, copy=False)
    h = h ^ (h >> np.uint32(16))
    h = h * np.uint32(0x85EBCA6B)
    h = h ^ (h >> np.uint32(13))
    h = h * np.uint32(0xC2B2AE35)
    h = h ^ (h >> np.uint32(16))
    return h


def _probe_slots(tok):
    hx = _mix32(tok.astype(np.uint32) ^ SEED)
    offs = np.arange(KP, dtype=np.uint32) * GOLD
    return (_mix32(hx[..., None] + offs) % np.uint32(NUM_SLOTS)).astype(np.int32)


def _split_multi_waits(nc, limit=1):
    """The nix-baked walrus rejects instructions with more than `limit`
    sem-waits ("Too many sync wait commands", CoreV3GenImpl setupSyncWait).
    Hoist extra waits onto single-wait NOPs preceding the instruction on
    the same engine (waiting earlier on the same engine is always safe)."""
    import concourse.mybir as mybir

    for fn in nc.m.functions:
        for bb in fn.blocks:
            new_insts = []
            for ins in bb.instructions:
                si = ins.sync_info
                if si is not None and len(si.on_wait) > limit:
                    waits = list(si.on_wait)
                    extra, keep = waits[:-limit], waits[-limit:]
                    for idx, w in enumerate(extra):
                        new_insts.append(mybir.InstNoOp(
                            name=f"{ins.name}-wsplit{idx}",
                            sync_info=mybir.SyncInfo(on_wait=[w], on_update=[]),
                            bass_nofuse=True,
                            engine=ins.engine,
                        ))
                    ins.sync_info = mybir.SyncInfo(
                        on_wait=keep, on_update=list(si.on_update))
                new_insts.append(ins)
            bb.instructions[:] = new_insts


def _strip_entry_barrier(nc):
    """Remove the entry-BB all-engine boot barrier and the const-tile
    memsets (walrus flags those consts as having no readers). The barrier
    only serializes engine boot: every real dependency in the body is
    carried by Tile-generated semaphores, and the event-semaphore
    barrier instances are self-resetting, so the exit barriers are
    unaffected. This lets each engine (notably the DMA-trigger engines)
    start its body work as soon as it boots instead of waiting ~3us for
    the slowest engine."""
    import concourse.mybir as mybir

    def _is_barrier(ins):
        if not isinstance(ins, (mybir.InstDrain, mybir.InstEventSemaphore)):
            return False
        si = ins.sync_info
        names = [w.ant_name for w in (si.on_wait if si else [])]
        names += [getattr(u, "ant_name", "") or ""
                  for u in (si.on_update if si else [])]
        return any(n.startswith("barrier_") for n in names) or not names

    bb = nc.m.functions[0].blocks[0]
    bb.instructions[:] = [
        ins for ins in bb.instructions
        if not (isinstance(ins, mybir.InstMemset) or _is_barrier(ins))
    ]




def _build(E, has_bias, wdt=None, split=True):
    import concourse.bass as bass
    import concourse.mybir as mybir
    from concourse.bass import MemorySpace
    from concourse.tile import TileContext

    if wdt is None:
        wdt = W_DTYPE
    f32 = mybir.dt.float32
    # float32r: same 4-byte fp32 layout, but the PE runs a single-pass
    # matmul (vs 2-pass FP32HI/FP32LO for plain fp32) at ~2x throughput
    # with slightly reduced internal precision.
    f32r = mybir.dt.float32r
    fw = mybir.dt.float16 if wdt == "f16" else f32r
    EC = E // 128
    nc = bass.Bass(monotonic_sem_count=0, enable_partition_id=False)
    # rows buffer: [E, D + B] — embedding row (D cols) | ct row (B cols),
    # merged so the whole phase-1 input arrives in ONE well-shaped DMA.
    rows = nc.declare_dram_parameter("rows", [E, D + B], fw, isOutput=False)
    wt = nc.declare_dram_parameter("wt", [D, VS], fw, isOutput=False)
    if has_bias:
        bias = nc.declare_dram_parameter("bias", [1, VS], f32, isOutput=False)
    out = nc.declare_dram_parameter("out", [B, VS], f32, isOutput=True)

    with TileContext(nc) as tc:
        with ExitStack() as ctx:
            const = ctx.enter_context(tc.tile_pool(name="const", bufs=1))
            rows_sb = const.tile([128, EC, D + B], fw)
            nc.gpsimd.dma_start(
                rows_sb[:], rows.rearrange("(n p) d -> p n d", p=128))
            if has_bias:
                bias_sb = const.tile([1, VS], f32)
                ones_sb = const.tile([1, B], f32)
                nc.sync.dma_start(bias_sb[:], bias[:])
                nc.any.memset(ones_sb[:], 1.0)

            wtp = ctx.enter_context(tc.tile_pool(name="wtp", bufs=16))
            obuf = ctx.enter_context(tc.tile_pool(name="obuf", bufs=NT))
            with tc.tile_pool(name="mpsum", bufs=NT, space=MemorySpace.PSUM) as mpsum:
                # PE warm-up: the HAM clock gate keeps the PE at 1.2 GHz
                # until it has seen ~3.4us of sustained matmul activity.
                # Run dummy matmuls on a zeroed tile while the first W
                # chunks are still in flight so the real matmuls start
                # at 2.4 GHz.
                NWARM = 14
                dumw = const.tile([128, 640], fw, name="dumw")
                nc.any.memset(dumw[:], 0.0)
                dps = mpsum.tile([128, 512], f32, name="ps")
                for i in range(NWARM):
                    nc.tensor.matmul(
                        dps[:],
                        dumw[:, :128],
                        dumw[:, 128:640],
                        start=True,
                        stop=True,
                    )

                # Phase 1: rT_k [128, 32] = rows[:, kchunk].T @ CT, k = 0..3
                # (reuses the same PSUM slots the big matmul uses later)
                rt_sb = []
                for k in range(KC):
                    rt_ps = mpsum.tile([128, B], f32, name="ps")
                    for e in range(EC):
                        nc.tensor.matmul(
                            rt_ps[:],
                            rows_sb[:, e, k * 128:(k + 1) * 128],
                            rows_sb[:, e, D:D + B],
                            start=(e == 0),
                            stop=(e == EC - 1),
                        )
                    rt_k = const.tile([128, B], fw, name=f"rt{k}")
                    nc.vector.tensor_copy(rt_k[:], rt_ps[:])
                    rt_sb.append(rt_k)

                # Phase 2: out[:, j*500:(j+1)*500] = rT.T @ wt_j (+ bias_j)
                # W stream: 512 KB transfers (4 KB contiguous per
                # partition) alternating between the sync- and scalar-
                # engine HWDGE queues so one queue's descriptor
                # generation hides under the other's data phase. All
                # tiles are SBUF-resident (bufs = #tiles) so the stream
                # never stalls on slot recycling. Output copies/stores
                # are interleaved into the last k-chunk.
                NW = (2 if wdt != "f16" else 4) * NTW
                NQT = VS // NW
                psums = [
                    mpsum.tile([B, NTW], f32, name="ps") for _ in range(NT)
                ]
                dma_engs = [nc.sync, nc.scalar]
                n_dma = 0
                for k in range(KC):
                    # Finer transfers on the last k-chunk: its completion
                    # semaphores gate the kernel tail, so smaller pieces
                    # start the final matmul/copy/store chain earlier.
                    if k == KC - 1:
                        col_chunks = [NW, NW // 2, NW // 4, NW // 4]
                    else:
                        col_chunks = [NW] * NQT
                    col0 = 0
                    for cw in col_chunks:
                        wq = wtp.tile([128, cw], fw, name="wq",
                                      padded_shape=[128, NW])
                        eng = dma_engs[n_dma % 2]
                        n_dma += 1
                        eng.dma_start(
                            wq[:],
                            wt[k * 128:(k + 1) * 128, col0:col0 + cw],
                        )
                        for jj in range(cw // NTW):
                            j = (col0 + jj * NTW) // NTW
                            nc.tensor.matmul(
                                psums[j][:],
                                rt_sb[k][:],
                                wq[:, jj * NTW:(jj + 1) * NTW],
                                start=(k == 0),
                                stop=(k == KC - 1 and not has_bias),
                            )
                            if k == KC - 1:
                                if has_bias:
                                    nc.tensor.matmul(
                                        psums[j][:],
                                        ones_sb[:],
                                        bias_sb[:, j * NTW:(j + 1) * NTW],
                                        start=False,
                                        stop=True,
                                    )
                                ob = obuf.tile([B, NTW], f32, name="ob")
                                if j == NT - 1:
                                    # Final tile: halve the copy across
                                    # DVE+ACT and store the halves on two
                                    # queues so the last store (and its
                                    # DRAM write receipt, which gates the
                                    # kernel tail) starts sooner.
                                    h = NTW // 2
                                    nc.vector.tensor_copy(
                                        ob[:, :h], psums[j][:, :h])
                                    nc.scalar.copy(
                                        ob[:, h:], psums[j][:, h:])
                                    nc.sync.dma_start(
                                        out[:, j * NTW:j * NTW + h],
                                        ob[:, :h])
                                    nc.gpsimd.dma_start(
                                        out[:, j * NTW + h:(j + 1) * NTW],
                                        ob[:, h:])
                                elif j % 2 == 0:
                                    nc.vector.tensor_copy(ob[:], psums[j][:])
                                    nc.gpsimd.dma_start(
                                        out[:, j * NTW:(j + 1) * NTW], ob[:])
                                else:
                                    nc.scalar.copy(ob[:], psums[j][:])
                                    nc.sync.dma_start(
                                        out[:, j * NTW:(j + 1) * NTW], ob[:])
                        col0 += cw
    if split:
        _split_multi_waits(nc)
        _strip_entry_barrier(nc)
    return nc


def _get_prog(E, has_bias):
    key = (E, has_bias, W_DTYPE)
    if key not in _prog_cache:
        _prog_cache[key] = _build(E, has_bias)
    return _prog_cache[key]


def _host_prep(x, emb_table):
    """Integer hash/match preprocessing -> packed rows [E, D + B]."""
    ts = np.arange(0, T - 1, 2)
    ts = ts[ts + 1 < T - 1]                      # [P]
    wslots = _probe_slots(x[:, ts])              # [B, P, K]
    qslots = _probe_slots(x[:, -1])              # [B, K]
    m = (wslots[:, :, None, :] == qslots[:, None, :, None]).sum(
        axis=(2, 3), dtype=np.int32)             # [B, P]
    bs, ps = np.nonzero(m)
    n_ent = len(bs)
    E = max(E_DEFAULT, ((n_ent + 127) // 128) * 128)
    rows = np.zeros((E, D + B), np.float32)      # emb row | ct row
    tok = x[:, ts + 1][bs, ps]                   # value tokens of hits
    rows[:n_ent, :D] = emb_table[tok]
    rows[np.arange(n_ent), D + bs] = m[bs, ps].astype(np.float32) / KP
    return rows


def kernel(x, emb_table, W, b):
    global LAST_RESULTS
    from concourse.bass_utils import run_bass_kernel_spmd

    x = np.asarray(x)
    emb_table = np.ascontiguousarray(np.asarray(emb_table, np.float32))
    W = np.asarray(W, np.float32)
    b = np.asarray(b, np.float32)

    rows = _host_prep(x, emb_table)
    has_bias = bool(np.any(b))
    wdt_np = np.float16 if W_DTYPE == "f16" else np.float32
    wt_full = np.ascontiguousarray(W.T.astype(wdt_np))   # [D, V]

    nc = _get_prog(rows.shape[0], has_bias)
    in_maps = []
    for c in range(NCORES):
        m = {
            "rows": rows.astype(wdt_np),
            "wt": np.ascontiguousarray(wt_full[:, c * VS:(c + 1) * VS]),
        }
        if has_bias:
            m["bias"] = np.ascontiguousarray(b[c * VS:(c + 1) * VS]).reshape(1, VS)
        in_maps.append(m)

    res = None
    for attempt in range(3):
        try:
            res = run_bass_kernel_spmd(
                nc, in_maps, core_ids=list(range(NCORES)))
            break
        except Exception:
            # The axon-tunneled device occasionally reports a transient
            # NRT_EXEC_UNIT_UNRECOVERABLE on back-to-back NEFF loads;
            # a re-dispatch on the next attempt succeeds.
            if attempt == 2:
                raise
            import time
            time.sleep(2.0)
    LAST_RESULTS = res

    logits = np.empty((B, V), np.float32)
    for c in range(NCORES):
        logits[:, c * VS:(c + 1) * VS] = res.results[c]["out"]
    return logits
